# revision 5
# baseline (speedup 1.0000x reference)
"""Trainium2 Bass kernel for CrossModalAttentionLayer.

Computes, for x:[64,1024,1024] y:[64,768] W_ch:[256,1024] b_ch:[256] W_y:[256,768]:
    y_k  = y @ W_y.T                      # [64, 256]
    x_k  = x @ W_ch.T + b_ch              # [64, 1024, 256]
    z    = tanh(x_k + y_k[:, None, :])
    attn = softmax(z, axis=-1)            # softmax over 256
    return attn.reshape(64*1024, 256)     # float32

Sharding: pure data parallel over the batch dim — 8 samples per NeuronCore.
Host-side prep casts x to fp16 and transposes it to [XC, rows] so the PE's
contraction dim lands on SBUF partitions with contiguous DMA lines.  The
matmul runs in fp16 (1 cycle/row on PE vs 4 for fp32) with fp32 PSUM
accumulation; tanh/exp/softmax run in fp32; the normalized output is stored
as fp16 (attn is in [1e-3, 1.1e-2], so fp16 adds ~5e-4 relative error) and
widened to fp32 on the host.  tanh output is in (-1,1) so exp never
overflows and softmax needs no max-subtraction.  The per-sample bias row
(y_k[b] + b_ch) is broadcast across partitions by a K=1 PE matmul with a
ones column.
"""

import os

import numpy as np

import concourse.bass as bass
import concourse.mybir as mybir
from concourse import bacc
import concourse.tile as tile
from concourse.bass_utils import run_bass_kernel_spmd

NCORES = 8
BS, N, XC, K, YS = 64, 1024, 1024, 256, 768
BP = BS // NCORES          # samples per core = 8
M = BP * N                 # rows per core = 8192
SP = 896                   # y-augmented contraction dim: 768 + 1 (ones) padded to 7*128

MSUB = 128                 # output-tile partition rows
GROUP = 1024               # rows loaded per DMA group (one sample)
PSUB = 4                   # m-subtiles accumulated per PSUM tile
NSUB = GROUP // MSUB       # 8 subtiles per group
NGROUPS = M // GROUP       # 8
CCH = XC // 128            # 8 contraction chunks
SCH = SP // 128            # 7 contraction chunks for the y path

F16 = mybir.dt.float16
F32 = mybir.dt.float32

LAST_RESULT = None         # BassKernelResults of the most recent run (for test harness)


def _emit(tc, nc, xt, wt, ya, wya, out, nrep=1):
    from contextlib import ExitStack

    with ExitStack() as ctx:
        singles = ctx.enter_context(tc.tile_pool(name="singles", bufs=1))
        xpool = ctx.enter_context(tc.tile_pool(name="x", bufs=3))
        ppool = ctx.enter_context(tc.tile_pool(name="psum", bufs=3, space="PSUM"))
        ypool = ctx.enter_context(tc.tile_pool(name="ypsum", bufs=1, space="PSUM"))
        zpool = ctx.enter_context(tc.tile_pool(name="z", bufs=3))
        epool = ctx.enter_context(tc.tile_pool(name="e", bufs=3))
        spool = ctx.enter_context(tc.tile_pool(name="s", bufs=4))
        opool = ctx.enter_context(tc.tile_pool(name="o", bufs=3))

        # --- replicated weights into SBUF ---
        wt_sb = singles.tile([128, CCH, K], F16)         # W_ch.T as [c128, chunk, k]
        nc.sync.dma_start(
            out=wt_sb, in_=wt[:, :].rearrange("(a p) k -> p a k", p=128)
        )
        ya_sb = singles.tile([128, SCH, BP], F32)        # [y; 1] transposed
        nc.sync.dma_start(
            out=ya_sb, in_=ya[:, :].rearrange("(a p) b -> p a b", p=128)
        )
        wya_sb = singles.tile([128, SCH, K], F32)        # [W_y.T; b_ch]
        nc.sync.dma_start(
            out=wya_sb, in_=wya[:, :].rearrange("(a p) k -> p a k", p=128)
        )
        ones_sb = singles.tile([1, MSUB], F16)
        nc.vector.memset(ones_sb, 1.0)

        # --- per-sample bias rows: bias[b,:] = y[b] @ W_y.T + b_ch  (fp32 PE) ---
        yk_ps = ypool.tile([BP, K], F32)
        for a in range(SCH):
            nc.tensor.matmul(
                yk_ps,
                lhsT=ya_sb[:, a, :],
                rhs=wya_sb[:, a, :],
                start=(a == 0),
                stop=(a == SCH - 1),
            )
        bias_sb = singles.tile([BP, K], F16)
        nc.scalar.copy(bias_sb, yk_ps)
        # all sample rows gathered onto partition 0 so the K=1 bias matmul can
        # slice its rhs at a free-dim offset
        bias_all = singles.tile([1, BP, K], F16)
        nc.sync.dma_start(out=bias_all, in_=bias_sb[:, :])

        # --- main loop over row groups (one sample per group) ---
        for rep in range(nrep):
            for g in range(NGROUPS):
                b = g // (N // GROUP)      # sample index (1024 rows per sample)
                m0 = g * GROUP

                xt_g = xpool.tile([128, CCH, GROUP], F16)
                nc.sync.dma_start(
                    out=xt_g,
                    in_=xt[:, :].rearrange("(a p) m -> p a m", p=128)[
                        :, :, m0 : m0 + GROUP
                    ],
                )

                for half in range(NSUB // PSUB):
                    psum_g = ppool.tile([128, PSUB, K], F32)
                    for psub in range(PSUB):
                        sub = half * PSUB + psub
                        for ch in range(CCH):
                            nc.tensor.matmul(
                                psum_g[:, psub, :],
                                lhsT=xt_g[:, ch, sub * MSUB : (sub + 1) * MSUB],
                                rhs=wt_sb[:, ch, :],
                                start=(ch == 0),
                                stop=False,
                            )
                        # += broadcast bias row (ones.T @ bias_row)
                        nc.tensor.matmul(
                            psum_g[:, psub, :],
                            lhsT=ones_sb,
                            rhs=bias_all[0:1, b, :],
                            start=False,
                            stop=True,
                        )

                    z_g = zpool.tile([128, PSUB, K], F32)
                    nc.scalar.activation(
                        z_g, psum_g, mybir.ActivationFunctionType.Tanh
                    )

                    e_g = epool.tile([128, PSUB, K], F32)
                    nc.scalar.activation(
                        e_g, z_g, mybir.ActivationFunctionType.Exp
                    )

                    s_g = spool.tile([128, PSUB, 1], F32)
                    nc.vector.reduce_sum(
                        out=s_g, in_=e_g, axis=mybir.AxisListType.X
                    )

                    r_g = spool.tile([128, PSUB, 1], F32)
                    nc.vector.reciprocal(r_g, s_g)

                    o_g = opool.tile([128, PSUB, K], F16)
                    for psub in range(PSUB):
                        nc.vector.tensor_scalar_mul(
                            o_g[:, psub, :], e_g[:, psub, :], r_g[:, psub, :]
                        )

                    h0 = m0 + half * PSUB * MSUB
                    nc.sync.dma_start(
                        out=out[h0 : h0 + PSUB * MSUB, :].rearrange(
                            "(a p) k -> p a k", p=128
                        ),
                        in_=o_g,
                    )


def build_bass(nrep=1):
    nc = bacc.Bacc()
    xt = nc.declare_dram_parameter("xt", [XC, M], F16, isOutput=False)
    wt = nc.declare_dram_parameter("wt", [XC, K], F16, isOutput=False)
    ya = nc.declare_dram_parameter("ya", [SP, BP], F32, isOutput=False)
    wya = nc.declare_dram_parameter("wya", [SP, K], F32, isOutput=False)
    out = nc.declare_dram_parameter("out", [M, K], F16, isOutput=True)
    with tile.TileContext(nc) as tc:
        _emit(tc, nc, xt, wt, ya, wya, out, nrep=nrep)
    nc.finalize()
    return nc


def prep_inputs(x, y, W_ch, b_ch, W_y):
    """Host-side shard + layout prep. Returns per-core input maps."""
    x = np.asarray(x, dtype=np.float32)
    y = np.asarray(y, dtype=np.float32)
    W_ch = np.asarray(W_ch, dtype=np.float32)
    b_ch = np.asarray(b_ch, dtype=np.float32)
    W_y = np.asarray(W_y, dtype=np.float32)

    wt_np = np.ascontiguousarray(W_ch.astype(np.float16).T)          # [XC, K]
    wya_np = np.zeros((SP, K), dtype=np.float32)
    wya_np[:YS] = W_y.T
    wya_np[YS] = b_ch

    in_maps = []
    for c in range(NCORES):
        xc = x[c * BP : (c + 1) * BP].reshape(M, XC).astype(np.float16)
        xt_c = np.ascontiguousarray(xc.T)                            # [XC, M]
        ya_c = np.zeros((SP, BP), dtype=np.float32)
        ya_c[:YS] = y[c * BP : (c + 1) * BP].T
        ya_c[YS] = 1.0
        in_maps.append({"xt": xt_c, "wt": wt_np, "ya": ya_c, "wya": wya_np})
    return in_maps


_NC_CACHE = None


def kernel(x, y, W_ch, b_ch, W_y):
    global _NC_CACHE, LAST_RESULT
    if _NC_CACHE is None:
        _NC_CACHE = build_bass()
    nc = _NC_CACHE
    in_maps = prep_inputs(x, y, W_ch, b_ch, W_y)
    kwargs = {}
    if os.environ.get("KERNEL_TRACE_DIR"):
        kwargs["tmpdir"] = os.environ["KERNEL_TRACE_DIR"]
    res = run_bass_kernel_spmd(nc, in_maps, list(range(NCORES)), **kwargs)
    LAST_RESULT = res
    return np.concatenate(
        [res.results[i]["out"].astype(np.float32) for i in range(NCORES)], axis=0
    )


# revision 10
# speedup vs baseline: 649.5393x; 649.5393x over previous
"""Trainium2 Bass kernel for CrossModalAttentionLayer.

Computes, for x:[64,1024,1024] y:[64,768] W_ch:[256,1024] b_ch:[256] W_y:[256,768]:
    y_k  = y @ W_y.T                      # [64, 256]
    x_k  = x @ W_ch.T + b_ch              # [64, 1024, 256]
    z    = tanh(x_k + y_k[:, None, :])
    attn = softmax(z, axis=-1)            # softmax over 256
    return attn.reshape(64*1024, 256)     # float32

Sharding: pure data parallel over the batch dim — 8 samples per NeuronCore.
Host-side prep casts x to fp16 and transposes it to [XC, rows] so the PE's
contraction dim lands on SBUF partitions with contiguous DMA lines.  The
matmul runs in fp16 (1 cycle/row on PE vs 4 for fp32) with fp32 PSUM
accumulation; tanh/exp/softmax run in fp32; the normalized output is stored
as fp16 (attn is in [1e-3, 1.1e-2], so fp16 adds ~5e-4 relative error) and
widened to fp32 on the host.  tanh output is in (-1,1) so exp never
overflows and softmax needs no max-subtraction.  The per-sample bias row
(y_k[b] + b_ch) is broadcast across partitions by a K=1 PE matmul with a
ones column.
"""

import os

import numpy as np

import concourse.bass as bass
import concourse.mybir as mybir
from concourse import bacc
import concourse.tile as tile
from concourse.bass_utils import run_bass_kernel_spmd

NCORES = 8
BS, N, XC, K, YS = 64, 1024, 1024, 256, 768
BP = BS // NCORES          # samples per core = 8
M = BP * N                 # rows per core = 8192
SP = 896                   # y-augmented contraction dim: 768 + 1 (ones) padded to 7*128

MSUB = 128                 # output-tile partition rows
GROUP = 512                # rows per processing unit (one DMA + one PSUM tile)
PSUB = GROUP // MSUB       # 4 m-subtiles per PSUM tile
NGROUPS = M // GROUP       # 16
CCH = XC // 128            # 8 contraction chunks
SCH = SP // 128            # 7 contraction chunks for the y path

F16 = mybir.dt.float16
F32 = mybir.dt.float32

LAST_RESULT = None         # BassKernelResults of the most recent run (for test harness)


def _emit(tc, nc, xt, wt, ya, wya, out, nrep=1, dyn_reps=1):
    from contextlib import ExitStack

    with ExitStack() as ctx:
        singles = ctx.enter_context(tc.tile_pool(name="singles", bufs=1))
        xpool = ctx.enter_context(tc.tile_pool(name="x", bufs=3))
        ppool = ctx.enter_context(tc.tile_pool(name="psum", bufs=3, space="PSUM"))
        ypool = ctx.enter_context(tc.tile_pool(name="ypsum", bufs=1, space="PSUM"))
        zpool = ctx.enter_context(tc.tile_pool(name="z", bufs=3))
        epool = ctx.enter_context(tc.tile_pool(name="e", bufs=3))
        spool = ctx.enter_context(tc.tile_pool(name="s", bufs=4))
        opool = ctx.enter_context(tc.tile_pool(name="o", bufs=3))

        # --- replicated weights into SBUF ---
        wt_sb = singles.tile([128, CCH, K], F16)         # W_ch.T as [c128, chunk, k]
        nc.sync.dma_start(
            out=wt_sb, in_=wt[:, :].rearrange("(a p) k -> p a k", p=128)
        )
        ya_sb = singles.tile([128, SCH, BP], F16)        # [y; 1] transposed
        nc.sync.dma_start(
            out=ya_sb, in_=ya[:, :].rearrange("(a p) b -> p a b", p=128)
        )
        wya_sb = singles.tile([128, SCH, K], F16)        # [W_y.T; b_ch]
        nc.sync.dma_start(
            out=wya_sb, in_=wya[:, :].rearrange("(a p) k -> p a k", p=128)
        )
        ones_sb = singles.tile([1, MSUB], F16)
        nc.vector.memset(ones_sb, 1.0)

        # --- per-sample bias rows: bias[b,:] = y[b] @ W_y.T + b_ch  (fp32 PE) ---
        yk_ps = ypool.tile([BP, K], F32)
        for a in range(SCH):
            nc.tensor.matmul(
                yk_ps,
                lhsT=ya_sb[:, a, :],
                rhs=wya_sb[:, a, :],
                start=(a == 0),
                stop=(a == SCH - 1),
            )
        bias_sb = singles.tile([BP, K], F16)
        nc.scalar.copy(bias_sb, yk_ps)
        # sample rows gathered onto partition 0, each duplicated twice, so one
        # K=1 matmul can seed two adjacent [128, K] PSUM regions with the bias
        bias_dup = singles.tile([1, BP, 2, K], F16)
        bsrc = bias_sb[:, :]
        # issued from ACT's HWDGE so the wait on the ACT copy above never
        # stalls SP's queue of bulk x loads
        nc.scalar.dma_start(
            out=bias_dup,
            in_=bass.AP(tensor=bsrc.tensor, offset=bsrc.offset,
                        ap=[bsrc.ap[0], [0, 2], bsrc.ap[1]]),
        )

        # --- main loop over 512-row units ---
        def body():
            for g in range(NGROUPS):
                b = g // (N // GROUP)      # sample index (1024 rows per sample)
                m0 = g * GROUP

                xt_g = xpool.tile([128, CCH, GROUP], F16)
                nc.sync.dma_start(
                    out=xt_g,
                    in_=xt[:, :].rearrange("(a p) m -> p a m", p=128)[
                        :, :, m0 : m0 + GROUP
                    ],
                )

                psum_g = ppool.tile([128, PSUB, K], F32)
                # seed two adjacent PSUM regions at once with the bias row
                for pair in range(PSUB // 2):
                    nc.tensor.matmul(
                        psum_g[:, 2 * pair : 2 * pair + 2, :],
                        lhsT=ones_sb,
                        rhs=bias_dup[0:1, b, :, :],
                        start=True,
                        stop=False,
                        skip_group_check=True,
                    )
                for psub in range(PSUB):
                    for ch in range(CCH):
                        nc.tensor.matmul(
                            psum_g[:, psub, :],
                            lhsT=xt_g[:, ch, psub * MSUB : (psub + 1) * MSUB],
                            rhs=wt_sb[:, ch, :],
                            start=False,
                            stop=(ch == CCH - 1),
                            skip_group_check=True,
                        )

                z_g = zpool.tile([128, PSUB, K], F32)
                nc.scalar.activation(
                    z_g, psum_g, mybir.ActivationFunctionType.Tanh
                )

                e_g = epool.tile([128, PSUB, K], F32)
                nc.scalar.activation(
                    e_g, z_g, mybir.ActivationFunctionType.Exp
                )

                s_g = spool.tile([128, PSUB, 1], F32)
                nc.vector.reduce_sum(
                    out=s_g, in_=e_g, axis=mybir.AxisListType.X
                )

                r_g = spool.tile([128, PSUB, 1], F32)
                nc.vector.reciprocal(r_g, s_g)

                o_g = opool.tile([128, PSUB, K], F16)
                for psub in range(PSUB):
                    nc.vector.tensor_scalar_mul(
                        o_g[:, psub, :], e_g[:, psub, :], r_g[:, psub, :]
                    )

                nc.sync.dma_start(
                    out=out[m0 : m0 + GROUP, :].rearrange(
                        "(a p) k -> p a k", p=128
                    ),
                    in_=o_g,
                )

        if dyn_reps > 1:
            with tc.For_i(0, dyn_reps, 1):
                body()
        else:
            for _ in range(nrep):
                body()


def build_bass(nrep=1, dyn_reps=1):
    nc = bacc.Bacc()
    xt = nc.declare_dram_parameter("xt", [XC, M], F16, isOutput=False)
    wt = nc.declare_dram_parameter("wt", [XC, K], F16, isOutput=False)
    ya = nc.declare_dram_parameter("ya", [SP, BP], F16, isOutput=False)
    wya = nc.declare_dram_parameter("wya", [SP, K], F16, isOutput=False)
    out = nc.declare_dram_parameter("out", [M, K], F16, isOutput=True)
    with tile.TileContext(nc) as tc:
        _emit(tc, nc, xt, wt, ya, wya, out, nrep=nrep, dyn_reps=dyn_reps)
    nc.finalize()
    return nc


def prep_inputs(x, y, W_ch, b_ch, W_y):
    """Host-side shard + layout prep. Returns per-core input maps."""
    x = np.asarray(x, dtype=np.float32)
    y = np.asarray(y, dtype=np.float32)
    W_ch = np.asarray(W_ch, dtype=np.float32)
    b_ch = np.asarray(b_ch, dtype=np.float32)
    W_y = np.asarray(W_y, dtype=np.float32)

    wt_np = np.ascontiguousarray(W_ch.astype(np.float16).T)          # [XC, K]
    wya_np = np.zeros((SP, K), dtype=np.float16)
    wya_np[:YS] = W_y.T.astype(np.float16)
    wya_np[YS] = b_ch.astype(np.float16)

    in_maps = []
    for c in range(NCORES):
        xc = x[c * BP : (c + 1) * BP].reshape(M, XC).astype(np.float16)
        xt_c = np.ascontiguousarray(xc.T)                            # [XC, M]
        ya_c = np.zeros((SP, BP), dtype=np.float16)
        ya_c[:YS] = y[c * BP : (c + 1) * BP].T.astype(np.float16)
        ya_c[YS] = 1.0
        in_maps.append({"xt": xt_c, "wt": wt_np, "ya": ya_c, "wya": wya_np})
    return in_maps


_NC_CACHE = None


def kernel(x, y, W_ch, b_ch, W_y):
    global _NC_CACHE, LAST_RESULT
    if _NC_CACHE is None:
        _NC_CACHE = build_bass()
    nc = _NC_CACHE
    in_maps = prep_inputs(x, y, W_ch, b_ch, W_y)
    kwargs = {}
    if os.environ.get("KERNEL_TRACE_DIR"):
        kwargs["tmpdir"] = os.environ["KERNEL_TRACE_DIR"]
    res = run_bass_kernel_spmd(nc, in_maps, list(range(NCORES)), **kwargs)
    LAST_RESULT = res
    return np.concatenate(
        [res.results[i]["out"].astype(np.float32) for i in range(NCORES)], axis=0
    )
